# revision 7
# baseline (speedup 1.0000x reference)
"""CrossCompressUnit kernel for TRN2 (8 NeuronCores, data-parallel over batch).

Math (collapsing the [B,D,D] outer product analytically):
    s1[b] = e[b,:] . w_vv      s2[b] = v[b,:] . w_ev
    s3[b] = e[b,:] . w_ve      s4[b] = v[b,:] . w_ee
    v_out[b,:] = v[b,:]*s1[b] + e[b,:]*s2[b] + b_vv
    e_out[b,:] = v[b,:]*s3[b] + e[b,:]*s4[b] + b_ee

Per-core plan (shard = 1024 rows as one [128, 1024] SBUF tile, row n*128+p at
partition p, chunk n):
  s-phase (per 128-row chunk): PE transposes v/e chunks (float32r bitcast:
  transpose is pure data movement, 1.5 vs 2 cycles/row), ScalarE copies the
  pair to SBUF, and two tiny fp32 matmuls against packed weight pairs append
  the four per-row dot products into one PSUM accumulator [128, 32]
  ([s2,s4,s1,s3] per chunk). One ScalarE copy lands all scalars in SBUF.

  elementwise phase (full width, 6 ops): strided views of s_all broadcast
  each per-row scalar across its chunk's 128 columns (stride-0 inner dim), so
  the whole shard is processed by two tensor_tensor multiplies + one fused
  scalar_tensor_tensor (+bias) per output, split across VectorE and GpSimd.

DMAs are whole-shard strided transfers on the sync sequencer. All constants
ride in one [128, 134] "aux" input; warmup ops sync each engine on the input
DMAs once so steady-state instructions keep <=1 sync wait.
"""

import sys

if "/opt/trn_rl_repo" not in sys.path:
    sys.path.insert(0, "/opt/trn_rl_repo")

from contextlib import ExitStack

import numpy as np

import concourse.bass as bass
import concourse.tile as tile
from concourse import bacc
from concourse import mybir
from concourse.bass_utils import run_bass_kernel_spmd

N_CORES = 8
B, D = 8192, 128
SHARD = B // N_CORES  # 1024 rows per core
NCHUNK = SHARD // 128  # 8 chunks of 128 rows

# aux layout (columns)
AUX_WV = 0   # [w_ev | w_ee] -> s2, s4
AUX_WE = 2   # [w_vv | w_ve] -> s1, s3
AUX_BVV = 4
AUX_BEE = 5
AUX_EYE = 6
AUX_COLS = 6 + D

F32 = mybir.dt.float32
F32R = mybir.dt.float32r
ALU = mybir.AluOpType

_CACHE: dict = {}


def _build_program() -> bass.Bass:
    nc = bacc.Bacc(
        "TRN2", target_bir_lowering=False, debug=False, num_devices=N_CORES
    )

    v_d = nc.dram_tensor("v", (SHARD, D), F32, kind="ExternalInput").ap()
    e_d = nc.dram_tensor("e", (SHARD, D), F32, kind="ExternalInput").ap()
    aux_d = nc.dram_tensor("aux", (D, AUX_COLS), F32, kind="ExternalInput").ap()
    vo_d = nc.dram_tensor("v_out", (SHARD, D), F32, kind="ExternalOutput").ap()
    eo_d = nc.dram_tensor("e_out", (SHARD, D), F32, kind="ExternalOutput").ap()

    with tile.TileContext(nc) as tc, ExitStack() as ctx:
        const = ctx.enter_context(tc.tile_pool(name="const", bufs=1))
        bigio = ctx.enter_context(tc.tile_pool(name="bigio", bufs=1))
        warm = ctx.enter_context(tc.tile_pool(name="warm", bufs=1, space="PSUM"))
        psum_t = ctx.enter_context(tc.tile_pool(name="psum_t", bufs=4, space="PSUM"))
        psum_s = ctx.enter_context(tc.tile_pool(name="psum_s", bufs=1, space="PSUM"))
        sb_t = ctx.enter_context(tc.tile_pool(name="sb_t", bufs=4))
        tmp = ctx.enter_context(tc.tile_pool(name="tmp", bufs=1))

        aux = const.tile([D, AUX_COLS], F32)
        nc.sync.dma_start(aux[:], aux_d)
        w_v = aux[:, AUX_WV : AUX_WV + 2]
        w_e = aux[:, AUX_WE : AUX_WE + 2]
        bvv = aux[:, AUX_BVV : AUX_BVV + 1]
        bee = aux[:, AUX_BEE : AUX_BEE + 1]
        eye = aux[:, AUX_EYE : AUX_EYE + D]

        # whole-shard loads (row (n*128 + p) -> partition p, chunk n)
        v_sb = bigio.tile([128, SHARD], F32)
        e_sb = bigio.tile([128, SHARD], F32)
        vo_sb = bigio.tile([128, SHARD], F32)
        eo_sb = bigio.tile([128, SHARD], F32)

        nc.sync.dma_start(
            v_sb[:].rearrange("p (n d) -> p n d", d=D),
            v_d.rearrange("(n p) d -> p n d", p=128),
        )
        nc.sync.dma_start(
            e_sb[:].rearrange("p (n d) -> p n d", d=D),
            e_d.rearrange("(n p) d -> p n d", p=128),
        )

        # Warmups: sync each compute engine once on the const/input DMAs so
        # steady-state instructions carry at most one semaphore wait.
        wpsum = warm.tile([128, D], F32)
        nc.tensor.transpose(wpsum[:], eye, eye)
        wsb = const.tile([128, 1], F32)
        nc.vector.tensor_copy(wsb[:], aux[:, AUX_BVV : AUX_BVV + 1])
        wsb2 = const.tile([128, 1], F32)
        nc.gpsimd.tensor_copy(wsb2[:], v_sb[:, 0:1])

        # ---- s phase: all 32 per-row dot products into one PSUM tile ------
        s_p = psum_s.tile([128, 4 * NCHUNK], F32)
        for c in range(NCHUNK):
            v_c = v_sb[:, c * D : (c + 1) * D]
            e_c = e_sb[:, c * D : (c + 1) * D]

            p_t = psum_t.tile([128, 2 * D], F32)
            nc.tensor.transpose(p_t[:, 0:D], v_c, eye)
            nc.tensor.transpose(p_t[:, D : 2 * D], e_c, eye)
            vt_et = sb_t.tile([128, 2 * D], F32)
            nc.scalar.copy(vt_et[:], p_t[:])

            nc.tensor.matmul(
                s_p[:, c * 4 : c * 4 + 2],
                lhsT=vt_et[:, 0:D], rhs=w_v, start=True, stop=True,
            )
            nc.tensor.matmul(
                s_p[:, c * 4 + 2 : c * 4 + 4],
                lhsT=vt_et[:, D : 2 * D], rhs=w_e, start=True, stop=True,
            )

        s_all = const.tile([128, 4 * NCHUNK], F32)
        nc.scalar.copy(s_all[:], s_p[:])

        # strided views: chunk c block is [s2, s4, s1, s3]; broadcast each
        # scalar over its chunk's 128 columns via a stride-0 inner dim
        def sview(off):
            return (
                s_all[:, off :: 4].unsqueeze(2).broadcast_to((128, NCHUNK, D))
            )

        s2v, s4v, s1v, s3v = sview(0), sview(1), sview(2), sview(3)
        v3 = v_sb[:].rearrange("p (n d) -> p n d", d=D)
        e3 = e_sb[:].rearrange("p (n d) -> p n d", d=D)
        vo3 = vo_sb[:].rearrange("p (n d) -> p n d", d=D)
        eo3 = eo_sb[:].rearrange("p (n d) -> p n d", d=D)

        # ---- elementwise phase: 6 full-width ops --------------------------
        t1 = tmp.tile([128, SHARD], F32)
        t2 = tmp.tile([128, SHARD], F32)
        t3 = tmp.tile([128, SHARD], F32)
        t4 = tmp.tile([128, SHARD], F32)
        t13 = t1[:].rearrange("p (n d) -> p n d", d=D)
        t23 = t2[:].rearrange("p (n d) -> p n d", d=D)
        t33 = t3[:].rearrange("p (n d) -> p n d", d=D)
        t43 = t4[:].rearrange("p (n d) -> p n d", d=D)

        nc.vector.tensor_tensor(t13, v3, s1v, ALU.mult)
        nc.gpsimd.tensor_tensor(t23, e3, s2v, ALU.mult)
        # v_out = (t1 + b_vv) + t2
        nc.vector.scalar_tensor_tensor(vo3, t13, bvv, t23, ALU.add, ALU.add)

        nc.gpsimd.tensor_tensor(t33, v3, s3v, ALU.mult)
        nc.vector.tensor_tensor(t43, e3, s4v, ALU.mult)
        # e_out = (t3 + b_ee) + t4
        nc.vector.scalar_tensor_tensor(eo3, t33, bee, t43, ALU.add, ALU.add)

        nc.sync.dma_start(
            vo_d.rearrange("(n p) d -> p n d", p=128),
            vo3,
        )
        nc.sync.dma_start(
            eo_d.rearrange("(n p) d -> p n d", p=128),
            eo3,
        )

    nc.compile()
    return nc


def _get_program() -> bass.Bass:
    if "nc" not in _CACHE:
        _CACHE["nc"] = _build_program()
    return _CACHE["nc"]


def _make_aux(w_vv, b_vv, w_ev, w_ve, w_ee, b_ee) -> np.ndarray:
    aux = np.zeros((D, AUX_COLS), dtype=np.float32)
    aux[:, AUX_WV + 0] = w_ev
    aux[:, AUX_WV + 1] = w_ee
    aux[:, AUX_WE + 0] = w_vv
    aux[:, AUX_WE + 1] = w_ve
    aux[:, AUX_BVV] = np.float32(np.asarray(b_vv).reshape(-1)[0])
    aux[:, AUX_BEE] = np.float32(np.asarray(b_ee).reshape(-1)[0])
    aux[:, AUX_EYE : AUX_EYE + D] = np.eye(D, dtype=np.float32)
    return aux


def kernel(v, e, w_vv, b_vv, w_ev, w_ve, w_ee, b_ee, _trace=False):
    v = np.ascontiguousarray(v, dtype=np.float32)
    e = np.ascontiguousarray(e, dtype=np.float32)
    assert v.shape == (B, D) and e.shape == (B, D)

    aux = _make_aux(w_vv, b_vv, w_ev, w_ve, w_ee, b_ee)
    in_maps = []
    for i in range(N_CORES):
        sl = slice(i * SHARD, (i + 1) * SHARD)
        in_maps.append({"v": v[sl], "e": e[sl], "aux": aux})

    nc = _get_program()
    res = run_bass_kernel_spmd(
        nc, in_maps, core_ids=list(range(N_CORES)), trace=_trace
    )

    v_out = np.concatenate([r["v_out"] for r in res.results], axis=0)
    e_out = np.concatenate([r["e_out"] for r in res.results], axis=0)
    if _trace:
        _CACHE["last_results"] = res
    return (v_out, e_out)


# revision 8
# speedup vs baseline: 1.1047x; 1.1047x over previous
"""CrossCompressUnit kernel for TRN2 (8 NeuronCores, data-parallel over batch).

Math (collapsing the [B,D,D] outer product analytically):
    s1[b] = e[b,:] . w_vv      s2[b] = v[b,:] . w_ev
    s3[b] = e[b,:] . w_ve      s4[b] = v[b,:] . w_ee
    v_out[b,:] = v[b,:]*s1[b] + e[b,:]*s2[b] + b_vv
    e_out[b,:] = v[b,:]*s3[b] + e[b,:]*s4[b] + b_ee

Per-core plan (shard = 1024 rows as one [128, 1024] SBUF tile, row n*128+p at
partition p, chunk n), processed in two software-pipelined halves:

  s-phase (per 2-chunk group): PE transposes four [128,128] chunks into one
  PSUM bank, ScalarE copies the group to SBUF, and tiny fp32 matmuls against
  packed weight pairs append the per-row dot products into a per-half PSUM
  accumulator ([s2,s4,s1,s3] per chunk). One ScalarE copy per half lands the
  scalars in SBUF. PE instructions are emitted back-to-back so the engine
  queue stays dense (per-instruction overhead dominates PE time).

  elementwise phase (per half, 6 ops): strided views of s_all broadcast each
  per-row scalar across its chunk's 128 columns (stride-0 inner dim); the
  half is processed by two tensor_tensor multiplies (GpSimd) + one multiply
  and one fused scalar_tensor_tensor (+bias) per output (VectorE).

I/O is split per half so the second half's compute overlaps the first half's
stores. All constants ride in one [128, 134] "aux" input; warmup ops sync
engines on the aux DMA once to keep steady-state sync waits low.
"""

import sys

if "/opt/trn_rl_repo" not in sys.path:
    sys.path.insert(0, "/opt/trn_rl_repo")

from contextlib import ExitStack

import numpy as np

import concourse.bass as bass
import concourse.tile as tile
from concourse import bacc
from concourse import mybir
from concourse.bass_utils import run_bass_kernel_spmd

N_CORES = 8
B, D = 8192, 128
SHARD = B // N_CORES  # 1024 rows per core
NCHUNK = SHARD // 128  # 8 chunks of 128 rows
HALF = SHARD // 2  # 512 columns per half
NGRP = 2  # chunk groups per half (2 chunks per group)

# aux layout (columns)
AUX_WV = 0   # [w_ev | w_ee] -> s2, s4
AUX_WE = 2   # [w_vv | w_ve] -> s1, s3
AUX_BVV = 4
AUX_BEE = 5
AUX_EYE = 6
AUX_COLS = 6 + D

F32 = mybir.dt.float32
ALU = mybir.AluOpType

_CACHE: dict = {}


def _build_program() -> bass.Bass:
    nc = bacc.Bacc(
        "TRN2", target_bir_lowering=False, debug=False, num_devices=N_CORES
    )

    v_d = nc.dram_tensor("v", (SHARD, D), F32, kind="ExternalInput").ap()
    e_d = nc.dram_tensor("e", (SHARD, D), F32, kind="ExternalInput").ap()
    aux_d = nc.dram_tensor("aux", (D, AUX_COLS), F32, kind="ExternalInput").ap()
    vo_d = nc.dram_tensor("v_out", (SHARD, D), F32, kind="ExternalOutput").ap()
    eo_d = nc.dram_tensor("e_out", (SHARD, D), F32, kind="ExternalOutput").ap()

    v3d = v_d.rearrange("(n p) d -> p n d", p=128)
    e3d = e_d.rearrange("(n p) d -> p n d", p=128)
    vo3d = vo_d.rearrange("(n p) d -> p n d", p=128)
    eo3d = eo_d.rearrange("(n p) d -> p n d", p=128)

    with tile.TileContext(nc) as tc, ExitStack() as ctx:
        const = ctx.enter_context(tc.tile_pool(name="const", bufs=1))
        bigio = ctx.enter_context(tc.tile_pool(name="bigio", bufs=1))
        warm = ctx.enter_context(tc.tile_pool(name="warm", bufs=1, space="PSUM"))
        psum_t = ctx.enter_context(tc.tile_pool(name="psum_t", bufs=3, space="PSUM"))
        psum_s = ctx.enter_context(tc.tile_pool(name="psum_s", bufs=2, space="PSUM"))
        sb_t = ctx.enter_context(tc.tile_pool(name="sb_t", bufs=4))
        sb_s = ctx.enter_context(tc.tile_pool(name="sb_s", bufs=2))
        tmp = ctx.enter_context(tc.tile_pool(name="tmp", bufs=2))

        aux = const.tile([D, AUX_COLS], F32)
        nc.sync.dma_start(aux[:], aux_d)
        w_v = aux[:, AUX_WV : AUX_WV + 2]
        w_e = aux[:, AUX_WE : AUX_WE + 2]
        bvv = aux[:, AUX_BVV : AUX_BVV + 1]
        bee = aux[:, AUX_BEE : AUX_BEE + 1]
        eye = aux[:, AUX_EYE : AUX_EYE + D]

        # whole-shard SBUF tiles; DMAs split per half for pipelining
        v_sb = bigio.tile([128, SHARD], F32)
        e_sb = bigio.tile([128, SHARD], F32)
        vo_sb = bigio.tile([128, SHARD], F32)
        eo_sb = bigio.tile([128, SHARD], F32)

        for h in range(2):
            ncs = slice(h * NCHUNK // 2, (h + 1) * NCHUNK // 2)
            fs = slice(h * HALF, (h + 1) * HALF)
            nc.sync.dma_start(
                v_sb[:, fs].rearrange("p (n d) -> p n d", d=D), v3d[:, ncs]
            )
            nc.sync.dma_start(
                e_sb[:, fs].rearrange("p (n d) -> p n d", d=D), e3d[:, ncs]
            )

        # Warmups: sync engines once on the aux/input DMAs.
        wpsum = warm.tile([128, D], F32)
        nc.tensor.transpose(wpsum[:], eye, eye)
        wsb = const.tile([128, 1], F32)
        nc.vector.tensor_copy(wsb[:], aux[:, AUX_BVV : AUX_BVV + 1])
        wsb2 = const.tile([128, 1], F32)
        nc.gpsimd.tensor_copy(wsb2[:], e_sb[:, 0:1])

        for h in range(2):
            # ---- s phase for this half: 4 chunks -> s_all_h [128, 16] -----
            s_p = psum_s.tile([128, 16], F32)
            for g in range(NGRP):
                c0 = h * 4 + g * 2
                p_t = psum_t.tile([128, 512], F32)
                for j in range(2):
                    c = c0 + j
                    v_c = v_sb[:, c * D : (c + 1) * D]
                    e_c = e_sb[:, c * D : (c + 1) * D]
                    nc.tensor.transpose(
                        p_t[:, j * 256 : j * 256 + D], v_c, eye
                    )
                    nc.tensor.transpose(
                        p_t[:, j * 256 + D : (j + 1) * 256], e_c, eye
                    )
                vt_et = sb_t.tile([128, 512], F32)
                nc.scalar.copy(vt_et[:], p_t[:])
                for j in range(2):
                    k = (g * 2 + j) * 4
                    nc.tensor.matmul(
                        s_p[:, k : k + 2],
                        lhsT=vt_et[:, j * 256 : j * 256 + D],
                        rhs=w_v, start=True, stop=True,
                    )
                    nc.tensor.matmul(
                        s_p[:, k + 2 : k + 4],
                        lhsT=vt_et[:, j * 256 + D : (j + 1) * 256],
                        rhs=w_e, start=True, stop=True,
                    )
            s_all = sb_s.tile([128, 16], F32)
            nc.scalar.copy(s_all[:], s_p[:])

            # ---- elementwise phase for this half (6 ops) ------------------
            def sview(off):
                return (
                    s_all[:, off :: 4]
                    .unsqueeze(2)
                    .broadcast_to((128, NCHUNK // 2, D))
                )

            s2v, s4v, s1v, s3v = sview(0), sview(1), sview(2), sview(3)
            fs = slice(h * HALF, (h + 1) * HALF)
            vh = v_sb[:, fs].rearrange("p (n d) -> p n d", d=D)
            eh = e_sb[:, fs].rearrange("p (n d) -> p n d", d=D)
            voh = vo_sb[:, fs].rearrange("p (n d) -> p n d", d=D)
            eoh = eo_sb[:, fs].rearrange("p (n d) -> p n d", d=D)

            t1 = tmp.tile([128, HALF], F32)
            t2 = tmp.tile([128, HALF], F32)
            t3 = tmp.tile([128, HALF], F32)
            t4 = tmp.tile([128, HALF], F32)
            t13 = t1[:].rearrange("p (n d) -> p n d", d=D)
            t23 = t2[:].rearrange("p (n d) -> p n d", d=D)
            t33 = t3[:].rearrange("p (n d) -> p n d", d=D)
            t43 = t4[:].rearrange("p (n d) -> p n d", d=D)

            nc.gpsimd.tensor_tensor(t23, eh, s2v, ALU.mult)
            nc.vector.tensor_tensor(t13, vh, s1v, ALU.mult)
            # v_out = (t1 + b_vv) + t2
            nc.vector.scalar_tensor_tensor(voh, t13, bvv, t23, ALU.add, ALU.add)
            nc.gpsimd.tensor_tensor(t33, vh, s3v, ALU.mult)
            nc.vector.tensor_tensor(t43, eh, s4v, ALU.mult)
            # e_out = (t3 + b_ee) + t4
            nc.vector.scalar_tensor_tensor(eoh, t33, bee, t43, ALU.add, ALU.add)

            ncs = slice(h * NCHUNK // 2, (h + 1) * NCHUNK // 2)
            nc.sync.dma_start(vo3d[:, ncs], voh)
            nc.sync.dma_start(eo3d[:, ncs], eoh)

    nc.compile()
    return nc


def _get_program() -> bass.Bass:
    if "nc" not in _CACHE:
        _CACHE["nc"] = _build_program()
    return _CACHE["nc"]


def _make_aux(w_vv, b_vv, w_ev, w_ve, w_ee, b_ee) -> np.ndarray:
    aux = np.zeros((D, AUX_COLS), dtype=np.float32)
    aux[:, AUX_WV + 0] = w_ev
    aux[:, AUX_WV + 1] = w_ee
    aux[:, AUX_WE + 0] = w_vv
    aux[:, AUX_WE + 1] = w_ve
    aux[:, AUX_BVV] = np.float32(np.asarray(b_vv).reshape(-1)[0])
    aux[:, AUX_BEE] = np.float32(np.asarray(b_ee).reshape(-1)[0])
    aux[:, AUX_EYE : AUX_EYE + D] = np.eye(D, dtype=np.float32)
    return aux


def kernel(v, e, w_vv, b_vv, w_ev, w_ve, w_ee, b_ee, _trace=False):
    v = np.ascontiguousarray(v, dtype=np.float32)
    e = np.ascontiguousarray(e, dtype=np.float32)
    assert v.shape == (B, D) and e.shape == (B, D)

    aux = _make_aux(w_vv, b_vv, w_ev, w_ve, w_ee, b_ee)
    in_maps = []
    for i in range(N_CORES):
        sl = slice(i * SHARD, (i + 1) * SHARD)
        in_maps.append({"v": v[sl], "e": e[sl], "aux": aux})

    nc = _get_program()
    res = run_bass_kernel_spmd(
        nc, in_maps, core_ids=list(range(N_CORES)), trace=_trace
    )

    v_out = np.concatenate([r["v_out"] for r in res.results], axis=0)
    e_out = np.concatenate([r["e_out"] for r in res.results], axis=0)
    if _trace:
        _CACHE["last_results"] = res
    return (v_out, e_out)
